# revision 1
# baseline (speedup 1.0000x reference)
"""BERT self-attention (no mask) on 8 TRN2 NeuronCores, head-parallel.

Full inputs in, full output out. Core c computes heads 2c and 2c+1, i.e.
output hidden columns [c*128, (c+1)*128). The host supplies X^T
([H, B*S], f32r) so no on-device transposes of X are needed; projections
consume X^T k-tiles straight from DMA. Matmul operands are float32r
(full-rate near-fp32 streaming). Attention is computed in transposed
layout (scores^T[k, q]) so the softmax denominator comes out of the PV
matmul for free via a ones-column appended to V. The device emits
UNNORMALIZED ctx^T plus denominators; the host divides, transposes, and
adds bv (softmax weights sum to 1, so +bv post-normalization is exact).
Projection (per batch) and attention (previous batch) are interleaved so
TensorE fills the gaps of the ACT-bound exp stream.
"""

import numpy as np

try:
    import concourse.bass as bass
except ImportError:  # toolchain not on sys.path in the caller's environment
    import sys
    sys.path.insert(0, "/opt/trn_rl_repo")
    import concourse.bass as bass
import concourse.bacc as bacc
import concourse.mybir as mybir
import concourse.tile as tile
from concourse.bass_utils import run_bass_kernel_spmd
from concourse.masks import make_identity

F32 = mybir.dt.float32
F32R = mybir.dt.float32r
BF16 = mybir.dt.bfloat16

B = 4
S = 2048
H = 1024
NH = 16
HD = 64
NSEQ = B * S  # 8192
NCORES = 8
CSLICE = H // NCORES  # 128 hidden cols per core = 2 heads
CHUNK = 512  # seq columns per projection chunk
KCH = H // 128  # 8 contraction tiles for projections
KT = S // 128  # 16 key tiles per (b, h)
QC = S // CHUNK  # 4 query chunks per (b, h)
EXPW = 1024  # exp tile width (2 psum banks)
VW = HD + 1  # V' tile width per key tile

_STATE = None


def _build():
    nc = bacc.Bacc("TRN2", target_bir_lowering=False, debug=False,
                   num_devices=NCORES)

    xT = nc.dram_tensor("xT", [H, NSEQ], F32R, kind="ExternalInput").ap()
    ws = {n: nc.dram_tensor(f"w{n}", [H, CSLICE], F32R,
                            kind="ExternalInput").ap()
          for n in "qkv"}
    bs = {n: nc.dram_tensor(f"b{n}", [CSLICE, 1], F32, kind="ExternalInput").ap()
          for n in "qk"}
    # unnormalized ctx^T + denominators: out[b*2+hl, d, s] with d==HD the
    # softmax denominator row; host divides and transposes.
    out = nc.dram_tensor("out", [B * 2, VW, S], F32, kind="ExternalOutput").ap()

    with tile.TileContext(nc) as tc:
        with (
            tc.tile_pool(name="persist", bufs=1) as persist,
            tc.tile_pool(name="qkvt", bufs=2) as qkvt_pool,
            tc.tile_pool(name="xt", bufs=3) as xt_pool,
            tc.tile_pool(name="vp", bufs=4) as vp_pool,
            tc.tile_pool(name="prob", bufs=12) as prob_pool,
            tc.tile_pool(name="cx", bufs=4) as cx_pool,
            tc.tile_pool(name="ppsum", bufs=2, space="PSUM") as ppsum,
            tc.tile_pool(name="spsum", bufs=2, space="PSUM") as spsum,
            tc.tile_pool(name="cpsum", bufs=2, space="PSUM") as cpsum,
        ):
            # f32r identity: walrus requires transpose operands to share a
            # transfer type when either is 32-bit; f32r streams at 1.5
            # cycles/row vs plain f32's 2.0.
            # 64x64 identity replicated in both partition halves, so
            # transposes of head-1 slices (base partition 64) have a
            # same-base permutation rhs.
            ident2_f = persist.tile([128, HD], F32)
            make_identity(nc, ident2_f[0:HD, :])
            make_identity(nc, ident2_f[HD:128, :])
            ident2 = persist.tile([128, HD], F32R)
            nc.vector.tensor_copy(ident2, ident2_f)
            ones = persist.tile([128, 1], F32)
            nc.vector.memset(ones, 1.0)

            # warm the PE p-state while the first DMAs are in flight:
            # cheap dummy matmuls on the identity (no DMA dependency),
            # on the ctx psum ring which attention won't touch for ~30us,
            # sized to end about when the first X^T tiles land.
            for i in range(26):
                wps = cpsum.tile([VW, CHUNK], F32, tag="ctx", name="warm")
                nc.tensor.matmul(wps[0:HD, 0:HD], ident2[0:HD, :],
                                 ident2[0:HD, :], start=True, stop=True)

            # one DMA per weight matrix: all 8 k-tiles land in a single
            # [128, 8*128] f32r tile via a 3D AP (partition = hid row mod
            # 128, free = [k-tile, out col]). Emitted lazily so chunk 0's
            # X^T loads get the HWDGE pipeline first.
            wt = {}  # weight k-tiles, lhsT layout [k 128, out 128]
            bt = {}

            def load_weights():
                for n in "qkv":
                    wall = persist.tile([128, KCH * CSLICE], F32R,
                                        tag=f"w{n}", name=f"w{n}")
                    nc.scalar.dma_start(
                        wall.rearrange("p (g c) -> p g c", g=KCH),
                        ws[n].rearrange("(g p) c -> p g c", g=KCH))
                    for kk in range(KCH):
                        wt[n, kk] = wall[:, kk * CSLICE:(kk + 1) * CSLICE]
                for n in "qk":
                    t = persist.tile([128, 1], F32, tag=f"b{n}", name=f"b{n}")
                    nc.scalar.dma_start(t, bs[n])
                    bt[n] = t

            def alloc_qkvT():
                # per-batch Q^T/K^T/V^T for this core's 2 heads: [128, 2048]
                return {n: qkvt_pool.tile([128, S], F32R,
                                          tag=f"{n}T", name=f"{n}T")
                        for n in "qkv"}

            def project_chunk_a(qkvT, ci, carry):
                    # 4 DMAs per chunk: each loads 2 contraction tiles
                    # [128, CHUNK] packed along the free dim via a 3D AP.
                    xts = []
                    for g in range(4):
                        xt = xt_pool.tile([128, 2 * CHUNK], F32R,
                                          tag=f"xt{g}", name=f"xt{g}")
                        src = xT[g * 256:(g + 1) * 256,
                                 ci * CHUNK:(ci + 1) * CHUNK]
                        nc.sync.dma_start(
                            xt.rearrange("p (g c) -> p g c", g=2),
                            src.rearrange("(g p) c -> p g c", g=2))
                        xts.append(xt)
                    carry[ci] = xts

            def project_chunk_b(qkvT, ci, carry, names="qkv"):
                    j = ci % QC
                    xts = carry.pop(ci)
                    for n in names:
                        ps = ppsum.tile([128, CHUNK], F32,
                                        tag="ps", name=f"ps{n}")
                        for kk in range(KCH):
                            nc.tensor.matmul(
                                ps, wt[n, kk],
                                xts[kk // 2][:, (kk % 2) * CHUNK:
                                             (kk % 2 + 1) * CHUNK],
                                start=(kk == 0), stop=(kk == KCH - 1))
                        dst = qkvT[n][:, j * CHUNK:(j + 1) * CHUNK]
                        if n == "v":
                            nc.vector.tensor_copy(dst, ps)
                        else:
                            nc.vector.tensor_scalar_add(dst, ps, bt[n])

            def prep_v(qkvT, hl):
                # all 16 V^T->V tile transposes go into one borrowed scores
                # psum tile, then a single strided DVE copy scatters them
                # into the VW-strided vp layout.
                p0 = hl * HD
                vT = qkvT["v"][p0:p0 + HD, :]
                vp = vp_pool.tile([128, KT * VW], F32R, tag="vp", name="vp")
                nc.vector.tensor_copy(
                    vp[:, HD::VW], ones.to_broadcast([128, KT]))
                vtp = spsum.tile([128, EXPW], F32, tag="s",
                                 name="vtp").bitcast(F32R)
                for kt in range(KT):
                    nc.tensor.transpose(
                        vtp[:, kt * HD:(kt + 1) * HD],
                        vT[:, kt * 128:(kt + 1) * 128],
                        ident2[p0:p0 + HD, :])
                nc.vector.tensor_copy(
                    vp.rearrange("p (kt d) -> p kt d", kt=KT)[:, :, 0:HD],
                    vtp.rearrange("p (kt d) -> p kt d", kt=KT))
                return vp

            def attend_qc(qkvT, b, hl, vp, qc):
                    p0 = hl * HD      # partition offset of this head
                    qT = qkvT["q"][p0:p0 + HD, :]
                    kTt = qkvT["k"][p0:p0 + HD, :]
                    ctx_ps = cpsum.tile([VW, CHUNK], F32,
                                        tag="ctx", name="ctx")
                    rhs_q = qT[:, qc * CHUNK:(qc + 1) * CHUNK]
                    for kp in range(KT // 2):  # pairs of key tiles
                        s_ps = spsum.tile([128, EXPW], F32, tag="s", name="s")
                        with tc.high_priority(offset=150):
                            for half in range(2):
                                kt = kp * 2 + half
                                nc.tensor.matmul(
                                    s_ps[:, half * CHUNK:(half + 1) * CHUNK],
                                    kTt[:, kt * 128:(kt + 1) * 128],
                                    rhs_q, start=True, stop=True)
                        pr = prob_pool.tile([128, EXPW], F32R,
                                            tag="pr", name="pr")
                        nc.scalar.activation(
                            pr, s_ps, mybir.ActivationFunctionType.Exp,
                            scale=1.0 / np.sqrt(float(HD)))
                        for half in range(2):
                            kt = kp * 2 + half
                            nc.tensor.matmul(
                                ctx_ps,
                                vp[:, kt * VW:(kt + 1) * VW],
                                pr[:, half * CHUNK:(half + 1) * CHUNK],
                                start=(kt == 0), stop=(kt == KT - 1))
                    cx = cx_pool.tile([VW, CHUNK], F32, tag="cx", name="cx")
                    with tc.high_priority(offset=150):
                        nc.vector.tensor_copy(cx, ctx_ps)
                    nc.sync.dma_start(
                        out[b * 2 + hl, :, qc * CHUNK:(qc + 1) * CHUNK], cx)

            def att_steps(qkvT, b, hl, vp):
                return [lambda qc=qc: attend_qc(qkvT, b, hl, vp, qc)
                        for qc in range(QC)]

            # software-pipelined emission: projection + V'-prep of batch
            # b+1 are emitted between the ACT-bound attention q-chunks of
            # batch b, giving the list scheduler adjacent independent work
            vps = {}
            qkvTs = {}
            carry = {}
            # lead-in with 2-chunk DMA lead: three chunks in flight (xt
            # ring depth 3) before the first projection consumes chunk 0
            qkvTs[0] = alloc_qkvT()
            project_chunk_a(qkvTs[0], 0, carry)
            load_weights()
            project_chunk_a(qkvTs[0], 1, carry)
            project_chunk_b(qkvTs[0], 0, carry)
            project_chunk_a(qkvTs[0], 2, carry)
            project_chunk_b(qkvTs[0], 1, carry)
            project_chunk_a(qkvTs[0], 3, carry)
            project_chunk_b(qkvTs[0], 2, carry)
            project_chunk_b(qkvTs[0], 3, carry)
            vps[0, 0] = prep_v(qkvTs[0], 0)
            vps[0, 1] = prep_v(qkvTs[0], 1)
            for b in range(B):
                if b == B - 1:
                    # last batch: no next-batch projection filler exists, so
                    # Q was held back (only K/V were projected ahead); emit
                    # Q chunk projections just-in-time, qc-major, as
                    # TensorE filler for the ACT-bound exp stream.
                    qkvT = qkvTs[b]

                    def qjit(qc, qkvT=qkvT, b=b):
                        project_chunk_b(qkvT, b * QC + qc, carry, names="q")

                    def aqc(hl, qc, qkvT=qkvT, b=b):
                        return lambda: attend_qc(qkvT, b, hl, vps[b, hl], qc)

                    def adma(qc, qkvT=qkvT, b=b):
                        return lambda: project_chunk_a(qkvT, b * QC + qc,
                                                       carry)

                    att = [
                        adma(1), lambda: qjit(0),
                        adma(2), lambda: qjit(1),
                        aqc(0, 0), aqc(1, 0),
                        adma(3), lambda: qjit(2),
                        aqc(0, 1), aqc(1, 1),
                        lambda: qjit(3),
                        aqc(0, 2), aqc(1, 2),
                        aqc(0, 3), aqc(1, 3),
                    ]
                else:
                    att = (att_steps(qkvTs[b], b, 0, vps[b, 0])
                           + att_steps(qkvTs[b], b, 1, vps[b, 1]))
                nxt = []
                if b + 1 < B:
                    names = "kv" if b + 1 == B - 1 else "qkv"
                    qkvTs[b + 1] = alloc_qkvT()
                    for ci in range(QC * (b + 1), QC * (b + 2)):
                        nxt.append(lambda ci=ci: project_chunk_a(
                            qkvTs[b + 1], ci, carry))
                        nxt.append(lambda ci=ci, names=names: project_chunk_b(
                            qkvTs[b + 1], ci, carry, names=names))
                    nxt.append(lambda: vps.__setitem__(
                        (b + 1, 0), prep_v(qkvTs[b + 1], 0)))
                    nxt.append(lambda: vps.__setitem__(
                        (b + 1, 1), prep_v(qkvTs[b + 1], 1)))
                    if b + 1 == B - 1:
                        nxt.append(lambda: project_chunk_a(
                            qkvTs[b + 1], QC * (b + 1), carry))
                # 8 att steps, up to 10 nxt steps: round-robin interleave
                order = list(att[:2])
                ai, ni = 2, 0
                while ai < len(att) or ni < len(nxt):
                    if ai < len(att):
                        order.append(att[ai]); ai += 1
                    if ni < len(nxt):
                        order.append(nxt[ni]); ni += 1
                    if ni < len(nxt) and len(nxt) - ni > len(att) - ai:
                        order.append(nxt[ni]); ni += 1
                for step in order:
                    step()

    nc.compile()
    return nc


def _get_nc():
    global _STATE
    if _STATE is None:
        _STATE = _build()
    return _STATE


def _in_maps(inputs):
    x = np.asarray(inputs["hidden_states"], dtype=np.float32).reshape(NSEQ, H)
    xTf = np.ascontiguousarray(x.T)  # [H, NSEQ]
    maps = []
    for c in range(NCORES):
        sl = slice(c * CSLICE, (c + 1) * CSLICE)
        m = {"xT": xTf}
        for n, wkey in (("q", "Wq"), ("k", "Wk"), ("v", "Wv")):
            m[f"w{n}"] = np.ascontiguousarray(
                np.asarray(inputs[wkey], dtype=np.float32)[:, sl])
        for n, bkey in (("q", "bq"), ("k", "bk")):
            m[f"b{n}"] = np.ascontiguousarray(
                np.asarray(inputs[bkey], dtype=np.float32)[sl].reshape(
                    CSLICE, 1))
        maps.append(m)
    return maps


def _assemble(results, inputs):
    bv = np.asarray(inputs["bv"], dtype=np.float32)
    full = np.empty((B, S, H), dtype=np.float32)
    for c in range(NCORES):
        o = results[c]["out"].reshape(B, 2, VW, S)
        ctx = o[:, :, :HD, :] / o[:, :, HD:HD + 1, :]  # [B, 2, HD, S]
        # -> [B, S, 2*HD]
        full[:, :, c * CSLICE:(c + 1) * CSLICE] = (
            ctx.transpose(0, 3, 1, 2).reshape(B, S, 2 * HD))
    full += bv.reshape(1, 1, H)
    return full


def _run(inputs, trace=False):
    nc = _get_nc()
    maps = _in_maps(inputs)
    last_err = None
    for attempt in range(3):
        try:
            res = run_bass_kernel_spmd(nc, maps,
                                       core_ids=list(range(NCORES)),
                                       trace=trace)
            return _assemble(res.results, inputs), res
        except Exception as e:  # transient NRT_EXEC_UNIT_UNRECOVERABLE
            last_err = e
            if attempt < 2:
                import time
                time.sleep(2.0)
    raise last_err


def kernel(**inputs):
    out, _ = _run(inputs, trace=False)
    return out


def run_traced(**inputs):
    out, res = _run(inputs, trace=True)
    return out, res



# revision 7
# speedup vs baseline: 1.1474x; 1.1474x over previous
"""BERT self-attention (no mask) on 8 TRN2 NeuronCores, head-parallel.

Full inputs in, full output out. Core c computes heads 2c and 2c+1 (output
hidden columns [c*128, (c+1)*128)). The host supplies X^T in bf16, so
projections consume k-tiles straight from DMA with no on-device transposes.

Layouts: Q^T/K^T are projected into [d, seq]; V is projected directly into
natural [seq, d] layout (X^T k-tile as the stationary operand, N=64 moving),
with a ones column appended per key tile so the PV matmul emits the softmax
denominator for free. Scores are computed transposed (s^T[k, q]); the PV
matmul is P-stationary: lhsT = pr[k, q-subtile], rhs = V[k, d+1], so ctx
lands in natural [q, d+1] layout and needs no transposes anywhere. All
matmul operands are bf16 (full rate at any moving size; fp32 psum). fp8
variants were tried and fail the error budget: softmax rows here are
concentrated (sum p^2 up to ~0.3) and raw scaled scores reach +-8.8, so
3-7% fp8 quantization of probs or V costs ~2e-2 output error on its own.

The ACT-bound exp stream (1 elem/cycle/lane) is relieved by computing a
small subset of tiles on DVE via a one-instruction Schraudolph exp2
(affine + f32->i16 convert, bitcast bf16, ~2% RMS); the subset size trades
ACT throughput against accuracy. Multi-region PSUM accumulation (8 V-proj
regions, 4 ctx regions per bank) issues start= on only the first matmul
per bank: start marks the whole 2KB zero region pending-zero, and each
region's first write then lands on still-pending bytes (overwrite).

The device emits UNNORMALIZED ctx[q, d] (with a uniform 2^-2 scale that
cancels at normalization) plus denominators; the host divides and adds bv
(softmax weights sum to 1, so +bv post-normalization is exact). Projection
of batch b+1 is interleaved between the attention q-chunks of batch b so
TensorE never starves while ACT/DVE chew on exp.
"""

import numpy as np

try:
    import concourse.bass as bass  # noqa: F401
except ImportError:  # toolchain not on sys.path in the caller's environment
    import sys
    sys.path.insert(0, "/opt/trn_rl_repo")
    import concourse.bass as bass  # noqa: F401
import concourse.bacc as bacc
import concourse.mybir as mybir
import concourse.tile as tile
import ml_dtypes
from concourse.bass_utils import run_bass_kernel_spmd
from concourse.masks import make_identity

F32 = mybir.dt.float32
BF16 = mybir.dt.bfloat16
I16 = mybir.dt.int16

B = 4
S = 2048
H = 1024
NH = 16
HD = 64
NSEQ = B * S  # 8192
NCORES = 8
CSLICE = H // NCORES  # 128 hidden cols per core = 2 heads
CHUNK = 512  # seq columns per projection chunk
KCH = H // 128  # 8 contraction tiles for projections
KT = S // 128  # 16 key tiles per (b, h)
QC = S // CHUNK  # 4 query chunks per (b, h)
EXPW = 1024  # exp tile width (2 psum banks)
VW = HD + 1  # V tile width per key tile (ones col for denominator)

LOG2E = float(np.log2(np.e))
LN2 = float(np.log(2.0))
SCHRAU_C = 0.043677
# pr = exp(s/8) * 2^-2 everywhere (uniform per row -> cancels at
# normalization; keeps headroom for the +-8.8 score tails in bf16).
ACT_BIAS = -2.0 * LN2
SCH_A = 16.0 * LOG2E  # = 128 * log2e / 8 (bf16 bit domain)
SCH_B = (125.0 - SCHRAU_C) * 128.0

# kp indices (per qc) whose exp runs on DVE instead of ACT. 4/32 trims the
# ACT stream to ~245us (matching the PE roofline) while the ~2% Schraudolph
# band on 1/8 of the keys stays within the error budget.
DVE_KP = [(2,), (5,), (1,), (4,)]

_STATE = None


def _build():
    nc = bacc.Bacc("TRN2", target_bir_lowering=False, debug=False,
                   num_devices=NCORES)

    xT = nc.dram_tensor("xT", [H, NSEQ], BF16, kind="ExternalInput").ap()
    wq = nc.dram_tensor("wwq", [H, CSLICE], BF16, kind="ExternalInput").ap()
    wk = nc.dram_tensor("wwk", [H, CSLICE], BF16, kind="ExternalInput").ap()
    wv = nc.dram_tensor("wwv", [H, CSLICE], BF16, kind="ExternalInput").ap()
    bq = nc.dram_tensor("bbq", [CSLICE, 1], F32, kind="ExternalInput").ap()
    bk = nc.dram_tensor("bbk", [CSLICE, 1], F32, kind="ExternalInput").ap()
    # unnormalized natural-layout ctx + denominator: out[b*2+hl, q, d] with
    # d==HD the softmax denominator; host divides and adds bv.
    out = nc.dram_tensor("out", [B * 2, S, VW], F32, kind="ExternalOutput").ap()

    with tile.TileContext(nc) as tc:
        with (
            tc.tile_pool(name="persist", bufs=1) as persist,
            tc.tile_pool(name="qkt", bufs=2) as qkt_pool,
            tc.tile_pool(name="vb", bufs=2) as vb_pool,
            tc.tile_pool(name="xt", bufs=3) as xt_pool,
            tc.tile_pool(name="pr", bufs=8) as pr_pool,
            tc.tile_pool(name="cx", bufs=4) as cx_pool,
            tc.tile_pool(name="ppsum", bufs=2, space="PSUM") as ppsum,
            tc.tile_pool(name="spsum", bufs=2, space="PSUM") as spsum,
            tc.tile_pool(name="cpsum", bufs=2, space="PSUM") as cpsum,
        ):
            identf = persist.tile([HD, HD], F32)
            make_identity(nc, identf)
            identb = persist.tile([HD, HD], BF16)
            nc.vector.tensor_copy(identb, identf)
            ebias = persist.tile([128, 1], F32)
            nc.vector.memset(ebias, ACT_BIAS)

            # warm the PE p-state while the first DMAs are in flight: cheap
            # dummy matmuls with no DMA dependency, on the ctx psum ring
            # which attention won't touch for a while.
            for i in range(40):
                wps = cpsum.tile([128, QC * VW], F32, tag="ctx", name="warm")
                nc.tensor.matmul(wps[0:HD, 0:HD], identb, identb,
                                 start=True, stop=True)

            # one DMA per weight matrix: all 8 k-tiles land in a single
            # [128, 8*128] tile via a 3D AP. Emitted lazily so chunk 0's
            # X^T loads get the HWDGE pipeline first.
            wt = {}
            bt = {}

            def load_weights():
                for n, src in (("q", wq), ("k", wk), ("v", wv)):
                    wall = persist.tile([128, KCH * CSLICE], BF16,
                                        tag=f"w{n}", name=f"w{n}")
                    nc.scalar.dma_start(
                        wall.rearrange("p (g c) -> p g c", g=KCH),
                        src.rearrange("(g p) c -> p g c", g=KCH))
                    wt[n] = wall
                for n, src in (("q", bq), ("k", bk)):
                    t = persist.tile([128, 1], F32, tag=f"b{n}", name=f"b{n}")
                    nc.scalar.dma_start(t, src)
                    bt[n] = t

            def alloc_qkT():
                # per-batch Q^T/K^T for this core's 2 heads: [128, 2048] bf16
                return {n: qkt_pool.tile([128, S], BF16,
                                         tag=f"{n}T", name=f"{n}T")
                        for n in "qk"}

            def alloc_vb():
                # natural-layout V per (hl): KT tiles of [128 seq, VW] bf16,
                # ones in column HD of each tile (PV denominator column).
                vs = []
                for hl in range(2):
                    v = vb_pool.tile([128, KT * VW], BF16,
                                     tag=f"vb{hl}", name=f"vb{hl}")
                    nc.gpsimd.memset(v[:, HD::VW], 1.0)
                    vs.append(v)
                return vs

            def project_chunk_a(ci, carry):
                # 4 DMAs per chunk: each loads 2 contraction k-tiles
                # [128, CHUNK] packed along the free dim via a 3D AP.
                xts = []
                for g in range(4):
                    xt = xt_pool.tile([128, 2 * CHUNK], BF16,
                                      tag=f"xt{g}", name=f"xt{g}")
                    src = xT[g * 256:(g + 1) * 256,
                             ci * CHUNK:(ci + 1) * CHUNK]
                    nc.sync.dma_start(
                        xt.rearrange("p (g c) -> p g c", g=2),
                        src.rearrange("(g p) c -> p g c", g=2))
                    xts.append(xt)
                carry[ci] = xts

            def project_chunk_b(qkT, vb, ci, carry, names="qkv"):
                j = ci % QC
                xts = carry.pop(ci)
                for n in names:
                    if n in "qk":
                        ps = ppsum.tile([128, CHUNK], F32,
                                        tag="ps", name=f"ps{n}")
                        wall = wt[n]
                        for kk in range(KCH):
                            nc.tensor.matmul(
                                ps, wall[:, kk * CSLICE:(kk + 1) * CSLICE],
                                xts[kk // 2][:, (kk % 2) * CHUNK:
                                             (kk % 2 + 1) * CHUNK],
                                start=(kk == 0), stop=(kk == KCH - 1))
                        dst = qkT[n][:, j * CHUNK:(j + 1) * CHUNK]
                        nc.vector.tensor_scalar_add(dst, ps, bt[n])
                    else:
                        # V natural layout: out[seq, d], stationary = X^T
                        # k-tile, moving = Wv columns (N=64). 8 regions in
                        # one psum bank: single start/stop for the bank.
                        vps = ppsum.tile([128, CHUNK], F32,
                                         tag="ps", name="psv")
                        for hl in range(2):
                            for sub in range(4):
                                reg = vps[:, hl * 256 + sub * HD:
                                          hl * 256 + (sub + 1) * HD]
                                for kk in range(KCH):
                                    nc.tensor.matmul(
                                        reg,
                                        xts[kk // 2][:, (kk % 2) * CHUNK
                                                     + sub * 128:
                                                     (kk % 2) * CHUNK
                                                     + (sub + 1) * 128],
                                        wt["v"][:, kk * CSLICE + hl * HD:
                                                kk * CSLICE + (hl + 1) * HD],
                                        start=(hl == 0 and sub == 0
                                               and kk == 0),
                                        stop=(hl == 1 and sub == 3
                                              and kk == KCH - 1),
                                        skip_group_check=True)
                        for hl in range(2):
                            dst = vb[hl][:, j * 4 * VW:(j + 1) * 4 * VW]
                            nc.vector.tensor_copy(
                                dst.rearrange("p (s d) -> p s d",
                                              s=4)[:, :, 0:HD],
                                vps[:, hl * 256:(hl + 1) * 256].rearrange(
                                    "p (s d) -> p s d", s=4))

            def attend_qc(qkT, vb, b, hl, qc):
                p0 = hl * HD
                qTh = qkT["q"][p0:p0 + HD, :]
                kTh = qkT["k"][p0:p0 + HD, :]
                v3 = vb[hl].rearrange("p (kt d) -> p kt d", kt=KT)
                ctx_ps = cpsum.tile([128, QC * VW], F32, tag="ctx", name="ctx")
                rhs_q = qTh[:, qc * CHUNK:(qc + 1) * CHUNK]
                dve_kp = DVE_KP[qc]
                for kp in range(KT // 2):  # pairs of key tiles
                    s_ps = spsum.tile([128, EXPW], F32, tag="s", name="s")
                    with tc.high_priority(offset=150):
                        for half in range(2):
                            kt = kp * 2 + half
                            nc.tensor.matmul(
                                s_ps[:, half * CHUNK:(half + 1) * CHUNK],
                                kTh[:, kt * 128:(kt + 1) * 128],
                                rhs_q, start=True, stop=True)
                    pr = pr_pool.tile([128, EXPW], BF16, tag="pr", name="pr")
                    if kp in dve_kp:
                        nc.vector.tensor_scalar(
                            pr.bitcast(I16), s_ps, SCH_A, SCH_B,
                            mybir.AluOpType.mult, mybir.AluOpType.add)
                    else:
                        nc.scalar.activation(
                            pr, s_ps, mybir.ActivationFunctionType.Exp,
                            bias=ebias, scale=0.125)
                    # 4 ctx accumulation regions in one psum bank: start
                    # only on the first matmul of the bank.
                    for half in range(2):
                        kt = kp * 2 + half
                        for sub in range(4):
                            nc.tensor.matmul(
                                ctx_ps[:, sub * VW:(sub + 1) * VW],
                                pr[:, half * CHUNK + sub * 128:
                                   half * CHUNK + (sub + 1) * 128],
                                v3[:, kt, :],
                                start=(kp == 0 and half == 0 and sub == 0),
                                stop=(kp == KT // 2 - 1 and half == 1
                                      and sub == 3),
                                skip_group_check=True)
                cx = cx_pool.tile([128, QC * VW], F32, tag="cx", name="cx")
                with tc.high_priority(offset=150):
                    nc.vector.tensor_copy(cx, ctx_ps)
                nc.sync.dma_start(
                    out[b * 2 + hl,
                        qc * CHUNK:(qc + 1) * CHUNK, :].rearrange(
                            "(s p) d -> p s d", s=4),
                    cx.rearrange("p (s d) -> p s d", s=4))

            def att_steps(qkT, vb, b):
                return [lambda hl=hl, qc=qc: attend_qc(qkT, vb, b, hl, qc)
                        for hl in range(2) for qc in range(QC)]

            # software-pipelined emission: projection of batch b+1 is
            # emitted between the attention q-chunks of batch b.
            qkTs = {}
            vbs = {}
            carry = {}
            qkTs[0] = alloc_qkT()
            vbs[0] = alloc_vb()
            project_chunk_a(0, carry)
            load_weights()
            project_chunk_a(1, carry)
            project_chunk_b(qkTs[0], vbs[0], 0, carry)
            project_chunk_a(2, carry)
            project_chunk_b(qkTs[0], vbs[0], 1, carry)
            project_chunk_a(3, carry)
            project_chunk_b(qkTs[0], vbs[0], 2, carry)
            project_chunk_b(qkTs[0], vbs[0], 3, carry)
            for b in range(B):
                if b == B - 1:
                    # last batch: no next-batch projection filler exists, so
                    # Q was held back (only K/V were projected ahead); emit
                    # Q chunk projections just-in-time as TensorE filler.
                    qkT, vb = qkTs[b], vbs[b]

                    def qjit(qc, qkT=qkT, vb=vb, b=b):
                        project_chunk_b(qkT, vb, b * QC + qc, carry,
                                        names="q")

                    def aqc(hl, qc, qkT=qkT, vb=vb, b=b):
                        return lambda: attend_qc(qkT, vb, b, hl, qc)

                    def adma(qc, b=b):
                        return lambda: project_chunk_a(b * QC + qc, carry)

                    att = [
                        adma(1), lambda: qjit(0),
                        adma(2), lambda: qjit(1),
                        aqc(0, 0), aqc(1, 0),
                        adma(3), lambda: qjit(2),
                        aqc(0, 1), aqc(1, 1),
                        lambda: qjit(3),
                        aqc(0, 2), aqc(1, 2),
                        aqc(0, 3), aqc(1, 3),
                    ]
                else:
                    att = att_steps(qkTs[b], vbs[b], b)
                nxt = []
                if b + 1 < B:
                    names = "kv" if b + 1 == B - 1 else "qkv"
                    qkTs[b + 1] = alloc_qkT()
                    vbs[b + 1] = alloc_vb()
                    for ci in range(QC * (b + 1), QC * (b + 2)):
                        nxt.append(lambda ci=ci: project_chunk_a(ci, carry))
                        nxt.append(
                            lambda ci=ci, names=names: project_chunk_b(
                                qkTs[b + 1], vbs[b + 1], ci, carry,
                                names=names))
                    if b + 1 == B - 1:
                        nxt.append(lambda: project_chunk_a(
                            QC * (b + 1), carry))
                # 8 att steps, up to 9 nxt steps: round-robin interleave
                order = list(att[:2])
                ai, ni = 2, 0
                while ai < len(att) or ni < len(nxt):
                    if ai < len(att):
                        order.append(att[ai]); ai += 1
                    if ni < len(nxt):
                        order.append(nxt[ni]); ni += 1
                    if ni < len(nxt) and len(nxt) - ni > len(att) - ai:
                        order.append(nxt[ni]); ni += 1
                for step in order:
                    step()

    nc.compile()
    return nc


def _get_nc():
    global _STATE
    if _STATE is None:
        _STATE = _build()
    return _STATE


def _in_maps(inputs):
    x = np.asarray(inputs["hidden_states"], dtype=np.float32).reshape(NSEQ, H)
    xTb = np.ascontiguousarray(x.T).astype(ml_dtypes.bfloat16)  # [H, NSEQ]
    maps = []
    for c in range(NCORES):
        sl = slice(c * CSLICE, (c + 1) * CSLICE)
        m = {"xT": xTb}
        for n, wkey in (("q", "Wq"), ("k", "Wk"), ("v", "Wv")):
            m[f"ww{n}"] = np.ascontiguousarray(
                np.asarray(inputs[wkey], dtype=np.float32)[:, sl]).astype(
                    ml_dtypes.bfloat16)
        for n, bkey in (("q", "bq"), ("k", "bk")):
            m[f"bb{n}"] = np.ascontiguousarray(
                np.asarray(inputs[bkey], dtype=np.float32)[sl].reshape(
                    CSLICE, 1))
        maps.append(m)
    return maps


def _assemble(results, inputs):
    bv = np.asarray(inputs["bv"], dtype=np.float32)
    full = np.empty((B, S, H), dtype=np.float32)
    for c in range(NCORES):
        o = results[c]["out"].reshape(B, 2, S, VW)
        ctx = o[:, :, :, :HD] / o[:, :, :, HD:HD + 1]  # [B, 2, S, HD]
        full[:, :, c * CSLICE:(c + 1) * CSLICE] = (
            ctx.transpose(0, 2, 1, 3).reshape(B, S, 2 * HD))
    full += bv.reshape(1, 1, H)
    return full


def _run(inputs, trace=False):
    nc = _get_nc()
    maps = _in_maps(inputs)
    last_err = None
    for attempt in range(3):
        try:
            res = run_bass_kernel_spmd(nc, maps,
                                       core_ids=list(range(NCORES)),
                                       trace=trace)
            return _assemble(res.results, inputs), res
        except Exception as e:  # transient NRT_EXEC_UNIT_UNRECOVERABLE
            last_err = e
            if attempt < 2:
                import time
                time.sleep(2.0)
    raise last_err


def kernel(**inputs):
    out, _ = _run(inputs, trace=False)
    return out


def run_traced(**inputs):
    out, res = _run(inputs, trace=True)
    return out, res
